# revision 24
# baseline (speedup 1.0000x reference)
"""Block-causal attention kernel for trn2, sharded over 8 NeuronCores.

Sharding: device d handles batch b = d // 4 and heads hA = 2*(d%4),
hB = hA + 1.  Each device computes its two heads' attention plus its
partial output projection partialT[c, t] (bf16); the host sums the 4
partials per batch and adds bo.

Design (v2):
- All SBUF operands bf16 (input x cast on host); PSUM f32.
- QK^T per 128-k-chunk: stationary kT[64, 128], moving qT[64, 512].
- exp on Act engine over merged [128, 2, Lg] (both heads, one instr).
- AV with *P-stationary* [128k, 128q] bf16 tiles and moving V'[128k, 65]
  (V columns + ones column for the softmax denominator): full 128x128
  PE utilization, 65-cycle matmuls, accumulated per q-block in PSUM
  via per-address has_written semantics (one start=True per bank/span).
- V^T computed directly via matmuls (stationary xT chunk, moving Wv).
- Normalization: per-q (partition) reciprocal * mul -> yn [q, d2].
- yn transposed back via PE transposes bitcast into the freed y2 bank.
- Output projection merges both heads (128-contraction) per c-block.
- Softmax without max-subtraction (scores ~N(0,1); exp safe in f32).
"""

import json

import numpy as np
import ml_dtypes

import concourse.bass as bass
import concourse.mybir as mybir
import concourse.tile as tile
from concourse.bass_utils import run_bass_kernel_spmd
from concourse.masks import make_identity
from concourse.vector_clock import ScopedClock

BF16 = mybir.dt.bfloat16
F32 = mybir.dt.float32

VP, B, C, H, W = 8, 2, 512, 16, 16
NH = 8
HD = C // NH  # 64
HWD = H * W  # 256 = block size
T = VP * HWD  # 2048
NCORES = 8
SCALE = 1.0 / np.sqrt(HD)

# ---------------------------------------------------------------------------
# Container workarounds (walrus in this image rejects >1 sync wait/update per
# instruction; Tile's tail drain carries many).
# ---------------------------------------------------------------------------


def _split_syncs(bir_bytes: bytes) -> bytes:
    j = json.loads(bir_bytes)
    changed = False
    for fn in j.get("functions", []):
        for bb in fn.get("blocks", []):
            out = []
            for inst in bb.get("instructions", []):
                si = inst.get("sync_info")
                if not si:
                    out.append(inst)
                    continue
                waits = si.get("on_wait") or []
                upds = si.get("on_update") or []
                if len(waits) > 1:
                    for i, w in enumerate(waits[:-1]):
                        out.append(
                            {
                                "debug": inst.get("debug", 0),
                                "engine": inst["engine"],
                                "ins": [],
                                "name": f"{inst['name']}_sw{i}",
                                "opcode": "EventSemaphore",
                                "outs": [],
                                "sync_info": {"on_update": [], "on_wait": [w]},
                            }
                        )
                    si["on_wait"] = waits[-1:]
                    changed = True
                out.append(inst)
                if len(upds) > 1:
                    si["on_update"] = upds[:1]
                    for i, u in enumerate(upds[1:]):
                        out.append(
                            {
                                "debug": inst.get("debug", 0),
                                "engine": inst["engine"],
                                "ins": [],
                                "name": f"{inst['name']}_su{i}",
                                "opcode": "EventSemaphore",
                                "outs": [],
                                "sync_info": {"on_update": [u], "on_wait": []},
                            }
                        )
                    changed = True
            bb["instructions"] = out
    return json.dumps(j).encode() if changed else bir_bytes


_patched = False


def _install_patches():
    global _patched
    if _patched:
        return
    _patched = True

    import concourse.bass2jax as bass2jax
    from concourse.bass_utils import compile_bir_kernel as _real_compile

    def patched_compile(bir_json, tmpdir, neff_name="file.neff"):
        return _real_compile(_split_syncs(bir_json), tmpdir, neff_name=neff_name)

    bass2jax.compile_bir_kernel = patched_compile

    def _drain_and_barrier(self, tick_clock, wait_clock):
        nc = self.nc
        drain_inst = nc.sync.drain()
        wait_clock.add_sem_waits(
            drain_inst.ins, ScopedClock({None: tick_clock.global_clock})
        )
        si = drain_inst.ins.sync_info
        waits = list(si.on_wait or [])
        if len(waits) > 1:
            si.on_wait = waits[:1]
            for w in waits[1:]:
                d2 = nc.sync.drain()
                d2.ins.sync_info = mybir.SyncInfo(on_wait=[w], on_update=[])
        nc.all_engine_barrier()
        assert self.sems is not None
        popped = nc._tile_sem_poison_stack.pop()
        assert popped is self._sem_poison
        nc.clear_and_free_semaphores(list(self.sems.allocated().values()))
        nc.all_engine_barrier()

    tile.TileContext._drain_and_barrier = _drain_and_barrier


# ---------------------------------------------------------------------------
# Device program (SPMD — same program on all 8 cores, different data)
# ---------------------------------------------------------------------------


def _build_program():
    _install_patches()
    nc = bass.Bass("TRN2", target_bir_lowering=False, debug=False, num_devices=NCORES)

    xT = nc.dram_tensor("xT", [C, T], BF16, kind="ExternalInput")
    wq = nc.dram_tensor("wq", [C, 128], BF16, kind="ExternalInput")
    wk = nc.dram_tensor("wk", [C, 128], BF16, kind="ExternalInput")
    wv = nc.dram_tensor("wv", [C, 128], BF16, kind="ExternalInput")
    # wo[0:64] = Wo rows of head A, wo[64:128] = head B  -> [128, C]
    wo = nc.dram_tensor("wo", [128, C], BF16, kind="ExternalInput")
    partialT = nc.dram_tensor("partialT", [C, T], F32, kind="ExternalOutput")

    EXP = mybir.ActivationFunctionType.Exp

    with tile.TileContext(nc) as tc:
        with (
            tc.tile_pool(name="persist", bufs=1) as pers,
            tc.tile_pool(name="work", bufs=2) as work,
            tc.tile_pool(name="ppool", bufs=16) as ppool,
            tc.tile_pool(name="stps", bufs=2, space="PSUM") as stps,
            tc.tile_pool(name="y2ps", bufs=1, space="PSUM") as y2ps,
            tc.tile_pool(name="pops", bufs=2, space="PSUM") as pops,
        ):
            # ---- persistent SBUF tiles
            xT_t = pers.tile([128, 4, T], BF16)
            wq_t = pers.tile([128, 4, 128], BF16)
            wk_t = pers.tile([128, 4, 128], BF16)
            wv_t = pers.tile([128, 4, 128], BF16)
            wo_t = pers.tile([128, C], BF16)
            F32R = mybir.dt.float32r
            qT_t = pers.tile([128, T], F32R)  # rows 0-63 head A, 64-127 head B
            kT_t = pers.tile([128, T], F32R)
            # V' per k-chunk: cols 0:64 = V_A, 64 = ones, 65:129 = V_B, 129 = ones
            v_t = pers.tile([128, 16, 130], BF16)
            ident = pers.tile([128, 128], BF16)

            make_identity(nc, ident)
            nc.vector.memset(v_t[:, :, 64:65], 1.0)
            nc.vector.memset(v_t[:, :, 129:130], 1.0)

            # input loads, span-major so span 0 compute can start early.
            # HWDGE (sync/scalar queues) is a single serialized device; SWDGE
            # (gpsimd) generates ~1.1us/DMA on the Pool engine but runs in
            # parallel with HWDGE — split span 0 across both, small weights
            # first on HWDGE, wo/wv late (needed only after span 0).
            nc.sync.dma_start(out=wq_t[:], in_=wq.rearrange("(c p) m -> p c m", p=128))
            nc.sync.dma_start(out=wk_t[:], in_=wk.rearrange("(c p) m -> p c m", p=128))
            for cc in (1, 3):
                nc.gpsimd.dma_start(
                    out=xT_t[:, cc, 0:512], in_=xT[cc * 128 : (cc + 1) * 128, 0:512]
                )
            for cc in (0, 2):
                nc.sync.dma_start(
                    out=xT_t[:, cc, 0:512], in_=xT[cc * 128 : (cc + 1) * 128, 0:512]
                )
            nc.gpsimd.dma_start(out=wv_t[:], in_=wv.rearrange("(c p) m -> p c m", p=128))
            nc.sync.dma_start(out=wo_t[:], in_=wo[:])
            for sp in range(1, 4):
                sl = slice(sp * 512, (sp + 1) * 512)
                for cc in range(4):
                    eng = nc.sync if cc % 2 == 0 else nc.gpsimd
                    eng.dma_start(
                        out=xT_t[:, cc, sl], in_=xT[cc * 128 : (cc + 1) * 128, sl]
                    )

            # prewarm the Exp activation table while DMAs run
            warm_in = work.tile([128, 1], F32, tag="warm_i")
            warm_out = work.tile([128, 1], F32, tag="warm_o")
            nc.vector.memset(warm_in[:], 0.0)
            nc.scalar.activation(warm_out[:], warm_in[:], EXP)
            # prewarm the PE p-state ramp (2.4GHz after 3us from first-busy)
            # with junk matmuls while the input DMAs stream in
            for _ in range(4):
                wmm = pops.tile([128, 512], F32, tag="pop")
                for r in range(4):
                    nc.tensor.matmul(
                        wmm[:, r * 128 : (r + 1) * 128], ident[:], ident[:],
                        start=(r == 0), stop=(r == 3), skip_group_check=True,
                    )

            # ---------- emission helpers ----------

            def qkv_q(sp):
                sl = slice(sp * 512, (sp + 1) * 512)
                ps = pops.tile([128, 512], F32, tag="pop")
                for cc in range(4):
                    nc.tensor.matmul(
                        ps[:], wq_t[:, cc, :], xT_t[:, cc, sl],
                        start=(cc == 0), stop=(cc == 3),
                    )
                nc.vector.tensor_copy(qT_t[:, sl], ps[:])

            def qkv_k(sp):
                sl = slice(sp * 512, (sp + 1) * 512)
                ps = pops.tile([128, 512], F32, tag="pop")
                for cc in range(4):
                    nc.tensor.matmul(
                        ps[:], wk_t[:, cc, :], xT_t[:, cc, sl],
                        start=(cc == 0), stop=(cc == 3),
                    )
                nc.vector.tensor_copy(kT_t[:, sl], ps[:])

            def qkv_v(sp):
                # direct V^T: out[k, d2] = sum_c x[c, k] Wv[c, d2], per k-chunk
                ps = pops.tile([128, 4, 128], F32, tag="pop")
                for i in range(4):
                    ksl = slice(sp * 512 + i * 128, sp * 512 + (i + 1) * 128)
                    for cc in range(4):
                        nc.tensor.matmul(
                            ps[:, i, :], xT_t[:, cc, ksl], wv_t[:, cc, :],
                            start=(i == 0 and cc == 0), stop=(cc == 3),
                            skip_group_check=True,
                        )
                j0 = sp * 4
                nc.vector.tensor_copy(
                    v_t[:, j0 : j0 + 4, 0:64], ps[:, :, 0:64]
                )
                nc.vector.tensor_copy(
                    v_t[:, j0 : j0 + 4, 65:129], ps[:, :, 64:128]
                )

            span_state = {}

            def normalize(sp):
                y2A, y2B = span_state[sp]["y2"]
                rec = work.tile([128, 2, 4, 1], F32, tag="rec")
                nc.vector.reciprocal(rec[:, 0, :, :], y2A[:, :, 64:65])
                nc.vector.reciprocal(rec[:, 1, :, :], y2B[:, :, 64:65])
                yn = work.tile([128, 4, 128], BF16, tag="yn")
                nc.vector.tensor_tensor(
                    out=yn[:, :, 0:64],
                    in0=y2A[:, :, 0:64],
                    in1=rec[:, 0, :, :].to_broadcast([128, 4, 64]),
                    op=mybir.AluOpType.mult,
                )
                nc.vector.tensor_tensor(
                    out=yn[:, :, 64:128],
                    in0=y2B[:, :, 0:64],
                    in1=rec[:, 1, :, :].to_broadcast([128, 4, 64]),
                    op=mybir.AluOpType.mult,
                )
                span_state[sp]["yn"] = yn

            def transpose_yt(sp):
                y2A, _ = span_state[sp]["y2"]
                yn = span_state[sp]["yn"]
                ytp = y2A[:].bitcast(BF16)  # [128, 4, 256] view of the bank
                for qb in range(4):
                    nc.tensor.matmul(
                        ytp[:, qb, 0:128], yn[:, qb, :], ident[:],
                        is_transpose=True,
                        start=(qb == 0), stop=(qb == 3),
                        skip_group_check=True,
                    )
                yt = work.tile([128, 512], BF16, tag="yt")
                nc.vector.tensor_copy(
                    yt[:].rearrange("p (q m) -> p q m", q=4), ytp[:, :, 0:128]
                )
                span_state[sp]["yt"] = yt

            def project(sp, half):
                yt = span_state[sp]["yt"]
                sl = slice(sp * 512, (sp + 1) * 512)
                for cc in (0, 1) if half == 0 else (2, 3):
                    po = pops.tile([128, 512], F32, tag="pop")
                    nc.tensor.matmul(
                        po[:], wo_t[:, cc * 128 : (cc + 1) * 128], yt[:],
                        start=True, stop=True,
                    )
                    stg = work.tile([128, 512], F32, tag=f"stg{cc % 2}")
                    nc.vector.tensor_copy(stg[:], po[:])
                    nc.sync.dma_start(
                        out=partialT[cc * 128 : (cc + 1) * 128, sl], in_=stg[:]
                    )

            def tail3_norm(h):
                # span 3 epilogue in halves: qb pair (0,1) finishes two
                # chunks before (2,3) — overlap its projection with the
                # last chunks instead of serializing after them.
                y2A, y2B = span_state[3]["y2"]
                qsl = slice(2 * h, 2 * h + 2)
                if h == 0:
                    span_state[3]["rec3"] = work.tile(
                        [128, 2, 4, 1], F32, tag="rec", name="rec3"
                    )
                    span_state[3]["yn3"] = work.tile(
                        [128, 4, 128], BF16, tag="yn", name="yn3"
                    )
                    span_state[3]["yt3"] = work.tile(
                        [128, 512], BF16, tag="yt", name="yt3"
                    )
                rec = span_state[3]["rec3"]
                yn = span_state[3]["yn3"]
                nc.vector.reciprocal(rec[:, 0, qsl, :], y2A[:, qsl, 64:65])
                nc.vector.reciprocal(rec[:, 1, qsl, :], y2B[:, qsl, 64:65])
                nc.vector.tensor_tensor(
                    out=yn[:, qsl, 0:64],
                    in0=y2A[:, qsl, 0:64],
                    in1=rec[:, 0, qsl, :].to_broadcast([128, 2, 64]),
                    op=mybir.AluOpType.mult,
                )
                nc.vector.tensor_tensor(
                    out=yn[:, qsl, 64:128],
                    in0=y2B[:, qsl, 0:64],
                    in1=rec[:, 1, qsl, :].to_broadcast([128, 2, 64]),
                    op=mybir.AluOpType.mult,
                )

            def tail3_proj(h):
                yn = span_state[3]["yn3"]
                yt = span_state[3]["yt3"]
                stx = stps.tile([128, 2, 512], F32, tag="st", name="stx")
                btc = stx[:].bitcast(BF16)
                for i in range(2):
                    nc.tensor.matmul(
                        btc[:, i, 0:128], yn[:, 2 * h + i, :], ident[:],
                        is_transpose=True, start=True, stop=True,
                        skip_group_check=True,
                    )
                hsl = slice(h * 256, (h + 1) * 256)
                nc.vector.tensor_copy(
                    yt[:, hsl].rearrange("p (q m) -> p q m", q=2), btc[:, :, 0:128]
                )
                csl = slice(1536 + h * 256, 1536 + (h + 1) * 256)
                for cc in range(4):
                    po = pops.tile([128, 512], F32, tag="pop")
                    nc.tensor.matmul(
                        po[:, 0:256], wo_t[:, cc * 128 : (cc + 1) * 128],
                        yt[:, hsl], start=True, stop=True,
                    )
                    stg = work.tile([128, 512], F32, tag=f"stg{cc % 2}")
                    nc.vector.tensor_copy(stg[:, 0:256], po[:, 0:256])
                    nc.sync.dma_start(
                        out=partialT[cc * 128 : (cc + 1) * 128, csl],
                        in_=stg[:, 0:256],
                    )

            # ---------- main loop ----------
            for sp in range(4):
                nj = 4 * sp + 4
                q0 = sp * 512
                p_tiles = {}
                extras = []
                if sp >= 1:
                    extras.append(lambda s=sp - 1: project(s, 0))
                    extras.append(lambda s=sp - 1: project(s, 1))
                if sp == 0:
                    qkv_q(0)
                    qkv_k(0)
                if sp <= 2:
                    extras.append(lambda s=sp + 1: qkv_q(s))
                    extras.append(lambda s=sp + 1: qkv_k(s))
                    extras.append(lambda s=sp + 1: qkv_v(s))

                def emit_qk(j):
                    off = 256 if j >= nj - 2 else 0
                    Lg = 512 - off
                    ksl = slice(j * 128, (j + 1) * 128)
                    qsl = slice(q0 + off, q0 + 512)
                    st = stps.tile([128, 2, 512], F32, tag="st")
                    nc.tensor.matmul(
                        st[:, 0, 0:Lg], kT_t[0:64, ksl], qT_t[0:64, qsl],
                        start=True, stop=True,
                    )
                    nc.tensor.matmul(
                        st[:, 1, 0:Lg], kT_t[64:128, ksl], qT_t[64:128, qsl],
                        start=True, stop=True,
                    )
                    pj = ppool.tile([128, 2, 512], BF16, tag="p")
                    nc.scalar.activation(pj[:, :, 0:Lg], st[:, :, 0:Lg], EXP)
                    p_tiles[j] = pj

                def emit_av(j):
                    y2A, y2B = span_state[sp]["y2"]
                    off = 256 if j >= nj - 2 else 0
                    pj = p_tiles[j]
                    qbs = range(2, 4) if off else range(4)
                    for qb in qbs:
                        c0 = qb * 128 - off
                        for h, y2 in ((0, y2A), (1, y2B)):
                            stop = (j == nj - 3 and qb < 2) or (
                                j == nj - 1 and qb >= 2
                            )
                            nc.tensor.matmul(
                                y2[:, qb, 0:65],
                                pj[:, h, c0 : c0 + 128],
                                v_t[:, j, h * 65 : h * 65 + 65],
                                start=(j == 0 and qb == 0),
                                stop=stop,
                                skip_group_check=True,
                            )

                for j in range(nj):
                    emit_qk(j)
                    if sp == 0 and j == 0:
                        qkv_v(0)
                    if j == 1:
                        if sp >= 1:
                            transpose_yt(sp - 1)
                        span_state[sp] = {
                            "y2": (
                                y2ps.tile(
                                    [128, 4, 128], F32, tag="y2A", name="y2A"
                                ),
                                y2ps.tile(
                                    [128, 4, 128], F32, tag="y2B", name="y2B"
                                ),
                            )
                        }
                    if j >= 1:
                        emit_av(j - 1)
                    if j >= 2 and extras:
                        extras.pop(0)()
                    if sp == 3 and j == nj - 2:
                        tail3_norm(0)
                emit_av(nj - 1)
                if sp < 3:
                    normalize(sp)
                for e in extras:
                    e()

            tail3_norm(1)
            tail3_proj(0)
            tail3_proj(1)
    return nc


_NC_CACHE = None


def _get_program():
    global _NC_CACHE
    if _NC_CACHE is None:
        _NC_CACHE = _build_program()
    return _NC_CACHE


def kernel(x, Wqkv, bqkv, bo=None, Wo=None, **kw):
    # accept arbitrary kw order; reference signature: x, Wqkv, bqkv, Wo, bo
    if Wo is None:
        Wo = kw["Wo"]
    if bo is None:
        bo = kw["bo"]
    x = np.asarray(x, dtype=np.float32)
    Wqkv = np.asarray(Wqkv, dtype=np.float32)
    bqkv = np.asarray(bqkv, dtype=np.float32)
    Wo = np.asarray(Wo, dtype=np.float32)
    bo = np.asarray(bo, dtype=np.float32)
    assert np.all(bqkv == 0.0), "nonzero bqkv not supported by this kernel build"

    bf = ml_dtypes.bfloat16
    nc = _get_program()
    in_maps = []
    for d in range(NCORES):
        b = d // 4
        hA = 2 * (d % 4)
        hB = hA + 1
        # xT [C, T]: t = (v, h, w)
        xT = np.ascontiguousarray(
            x[:, b].transpose(1, 0, 2, 3).reshape(C, T)
        ).astype(bf)
        qcols = np.r_[hA * HD : (hA + 1) * HD, hB * HD : (hB + 1) * HD]
        in_maps.append(
            {
                "xT": xT,
                "wq": np.ascontiguousarray(Wqkv[:, qcols] * SCALE).astype(bf),
                "wk": np.ascontiguousarray(Wqkv[:, C + qcols]).astype(bf),
                "wv": np.ascontiguousarray(Wqkv[:, 2 * C + qcols]).astype(bf),
                "wo": np.ascontiguousarray(Wo[qcols, :]).astype(bf),
            }
        )

    res = run_bass_kernel_spmd(nc, in_maps, core_ids=list(range(NCORES)))
    global _LAST_RES
    _LAST_RES = res

    out = np.empty((VP, B, C, H, W), dtype=np.float32)
    for b in range(B):
        acc = np.zeros((C, T), dtype=np.float32)
        for d in range(b * 4, b * 4 + 4):
            acc += res.results[d]["partialT"]
        acc += bo[:, None]
        out[:, b] = acc.reshape(C, VP, H, W).transpose(1, 0, 2, 3)
    return out
